# revision 1
# baseline (speedup 1.0000x reference)
"""GNN message-passing kernel for 8 Trainium2 NeuronCores.

Computes (reference semantics):
    h = tanh(node_feat @ w_n2l + b_n2l)
    for lv in range(3):
        conv = (h @ conv_w[lv] + conv_b[lv]).reshape(N, 4, D)
        msgs[e] = segment_sum(conv[:, e, :][src_e], dst_e, N)
        msg = tanh(concat_e(msgs))
        h = tanh(msg @ merge_w[lv] + merge_b[lv] + h)

Key algebraic rewrite: conv is linear in h, so
    segment_sum(conv_e(h)[src], dst) = segment_sum(h[src], dst) @ W_e + deg_e * b_e
which lets us gather 128-wide h rows (bf16) instead of 512-wide conv rows.

Distribution: nodes sharded over 8 cores (12500 each). Weights replicated.
Each level: AllGather bf16 h -> per-core full copy in DRAM; per-core
dma_gather of source rows for locally-owned edges; scatter realized as
one-hot matmul into PSUM per 128-node dst block.
"""

import math
import numpy as np
import ml_dtypes

BF16 = ml_dtypes.bfloat16

# ----------------------------------------------------------------------------
# configuration
# ----------------------------------------------------------------------------


class Cfg:
    def __init__(self, n_nodes=100000, n_cores=8, ept=1600000, piece_chunks=32):
        self.N = n_nodes
        self.NC = n_cores
        self.ET = 4          # edge types
        self.LV = 3          # levels
        self.D = 128         # feature dim (latent == feats == 128)
        self.EPT = ept       # edges per type
        self.S = self.N // self.NC            # real nodes per core
        self.SP = ((self.S + 127) // 128) * 128   # padded nodes per core
        self.W = self.SP // 128               # dst blocks per core
        self.REG = self.SP + 1                # shard rows incl. zero row
        self.NREG = self.NC // 2              # src core-pair regions
        self.REG_ROWS = 2 * self.REG          # rows per pair region
        assert self.REG_ROWS - 1 <= 32767, "int16 gather index limit"
        self.PIECE_CH = piece_chunks          # chunks per gather piece
        self.PIECE = self.PIECE_CH * 128      # slots per gather piece


# ----------------------------------------------------------------------------
# CPU preprocessing: edge bucketing, capacities, index/offset arrays
# ----------------------------------------------------------------------------


def preprocess_edges(cfg, edge_src, edge_dst):
    """Bucket edges by (dst core, scp stream, dst block, edge type).

    Returns a plan shared by all cores (capacities, piece table) plus
    per-core idx / dstoff arrays.
    """
    NC, ET, W, S = cfg.NC, cfg.ET, cfg.W, cfg.S
    NREG = cfg.NREG

    # group sizes m[c, e, scp, w]
    m = np.zeros((NC, ET, NREG, W), dtype=np.int64)
    per_core_edges = [[None] * ET for _ in range(NC)]  # (scp, w, srcidx, dstoff)
    for e in range(ET):
        src = np.asarray(edge_src[e], dtype=np.int64)
        dst = np.asarray(edge_dst[e], dtype=np.int64)
        owner = dst // S
        dl = dst - owner * S
        w = dl >> 7
        scp = src // (2 * S)
        src_in_pair = src - scp * (2 * S)
        idx16 = (src_in_pair // S) * cfg.REG + (src_in_pair % S)
        doff = dl & 127
        key = (owner * NREG + scp) * W + w
        cnt = np.bincount(key, minlength=NC * NREG * W).reshape(NC, NREG, W)
        m[:, e] = cnt
        order = np.argsort(key, kind="stable")
        ksorted = key[order]
        bounds = np.searchsorted(ksorted, np.arange(NC) * NREG * W)
        bounds = np.append(bounds, len(ksorted))
        for c in range(NC):
            sl = order[bounds[c]:bounds[c + 1]]
            per_core_edges[c][e] = (
                scp[sl], w[sl], idx16[sl].astype(np.int16), doff[sl].astype(np.int16)
            )

    # capacities (chunks of 128) per (e, scp, w): max over cores
    K = np.ceil(m.max(axis=0) / 128.0).astype(np.int64)  # [ET, NREG, W]

    # stream layout per scp: slots ordered by (w, e); group (e,scp,w) gets
    # K[e,scp,w]*128 slots.
    group_slot_start = np.zeros((ET, NREG, W), dtype=np.int64)
    stream_len = np.zeros(NREG, dtype=np.int64)
    for scp in range(NREG):
        off = 0
        for w in range(W):
            for e in range(ET):
                group_slot_start[e, scp, w] = off
                off += K[e, scp, w] * 128
        stream_len[scp] = off

    # piece table: per scp stream, pieces of cfg.PIECE slots (last short)
    pieces = []  # (scp, slot_start, n_slots, global_piece_index)
    piece_of_stream = []
    for scp in range(NREG):
        start_list = []
        off = 0
        while off < stream_len[scp]:
            n = min(cfg.PIECE, stream_len[scp] - off)
            n = ((n + 127) // 128) * 128
            start_list.append((len(pieces), off, n))
            pieces.append((scp, off, n))
            off += n
        piece_of_stream.append(start_list)

    npieces = len(pieces)
    idx_cols = cfg.PIECE // 16
    off_cols = cfg.PIECE_CH

    # per-core arrays
    idx_arrs = []
    off_arrs = []
    deg_arrs = []
    for c in range(NC):
        idx_flat = [np.full(stream_len[scp], cfg.SP, dtype=np.int16)
                    for scp in range(NREG)]  # pad -> zero row (row SP of shard 0)
        off_flat = [np.zeros(stream_len[scp], dtype=np.int16) for scp in range(NREG)]
        deg = np.zeros((ET, cfg.SP), dtype=np.float32)
        for e in range(ET):
            escp, ew, eidx, edoff = per_core_edges[c][e]
            # position within group via stable sort on (scp, w)
            gkey = escp * W + ew
            order = np.argsort(gkey, kind="stable")
            gs = gkey[order]
            # rank within group
            grp_start_pos = np.searchsorted(gs, gs, side="left")
            rank = np.arange(len(gs)) - grp_start_pos
            slot = group_slot_start[e, escp[order], ew[order]] + rank
            for scp in range(NREG):
                msk = escp[order] == scp
                idx_flat[scp][slot[msk]] = eidx[order][msk]
                off_flat[scp][slot[msk]] = edoff[order][msk]
            dln = ew * 128 + edoff
            deg[e] = np.bincount(dln, minlength=cfg.SP).astype(np.float32)
        # wrap into DMA layouts
        idx_arr = np.zeros((npieces, 128, idx_cols), dtype=np.int16)
        off_arr = np.zeros((npieces, 128, off_cols), dtype=np.int16)
        for scp in range(NREG):
            for (pid, off0, n) in piece_of_stream[scp]:
                chunk = idx_flat[scp][off0:off0 + n]
                wrapped = chunk.reshape(-1, 16).T          # [16, n/16]
                idx_arr[pid, :, : n // 16] = np.tile(wrapped, (8, 1))
                oc = off_flat[scp][off0:off0 + n].reshape(-1, 128).T  # [128, n/128]
                off_arr[pid, :, : n // 128] = oc
        idx_arrs.append(idx_arr)
        off_arrs.append(off_arr)
        deg_arrs.append(np.ascontiguousarray(deg.reshape(ET, cfg.W, 1, 128)))

    plan = {
        "K": K,
        "group_slot_start": group_slot_start,
        "stream_len": stream_len,
        "pieces": pieces,
        "piece_of_stream": piece_of_stream,
        "npieces": npieces,
        "idx_cols": idx_cols,
        "off_cols": off_cols,
    }
    return plan, idx_arrs, off_arrs, deg_arrs


# ----------------------------------------------------------------------------
# program builder
# ----------------------------------------------------------------------------


def build_program(cfg, plan):
    from concourse import bass, bacc, tile, mybir

    f32 = mybir.dt.float32
    b16 = mybir.dt.bfloat16
    i16 = mybir.dt.int16
    Tanh = mybir.ActivationFunctionType.Tanh
    Copy = mybir.ActivationFunctionType.Copy

    NC, ET, W, LV = cfg.NC, cfg.ET, cfg.W, cfg.LV
    NREG = cfg.NREG
    SP, REG = cfg.SP, cfg.REG
    K = plan["K"]
    group_slot_start = plan["group_slot_start"]
    pieces = plan["pieces"]
    piece_of_stream = plan["piece_of_stream"]
    npieces = plan["npieces"]
    idx_cols = plan["idx_cols"]
    off_cols = plan["off_cols"]

    nc = bacc.Bacc("TRN2", target_bir_lowering=False, debug=False,
                   num_devices=NC)

    # ---- external tensors ----
    nf_t = nc.dram_tensor("node_feat_t", [128, SP], f32, kind="ExternalInput")
    idx_d = nc.dram_tensor("idx", [npieces, 128, idx_cols], i16, kind="ExternalInput")
    off_d = nc.dram_tensor("dstoff", [npieces, 128, off_cols], i16, kind="ExternalInput")
    deg_d = nc.dram_tensor("deg", [ET, W, 1, 128], f32, kind="ExternalInput")
    w_n2l_d = nc.dram_tensor("w_n2l", [128, 128], f32, kind="ExternalInput")
    b_n2l_d = nc.dram_tensor("b_n2l", [1, 128], f32, kind="ExternalInput")
    convw_d = nc.dram_tensor("conv_w", [LV, 128, ET * 128], f32, kind="ExternalInput")
    convb_d = nc.dram_tensor("conv_b", [LV, 1, ET * 128], f32, kind="ExternalInput")
    mw_d = nc.dram_tensor("merge_w", [LV, 128, ET * 128], f32, kind="ExternalInput")
    mb_d = nc.dram_tensor("merge_b", [LV, 1, 128], f32, kind="ExternalInput")
    out_d = nc.dram_tensor("out", [SP, 128], f32, kind="ExternalOutput")

    with tile.TileContext(nc) as tc:
        with (
            tc.tile_pool(name="dram", bufs=1, space="DRAM") as dramp,
            tc.tile_pool(name="dram2", bufs=2, space="DRAM") as dramp2,
            tc.tile_pool(name="const", bufs=1) as constp,
            tc.tile_pool(name="wts", bufs=2) as wtsp,
            tc.tile_pool(name="gather", bufs=2) as gatherp,
            tc.tile_pool(name="ponehot", bufs=2) as ponep,
            tc.tile_pool(name="idxp", bufs=2) as idxp,
            tc.tile_pool(name="small", bufs=3) as smallp,
            tc.tile_pool(name="mt", bufs=2) as mtp,
            tc.tile_pool(name="psum_s", bufs=3, space="PSUM") as psum_s_pool,
            tc.tile_pool(name="psum_t", bufs=2, space="PSUM") as psum_t_pool,
            tc.tile_pool(name="psum_hn", bufs=2, space="PSUM") as psum_hn_pool,
        ):
            # ---- DRAM intermediates ----
            shard = dramp.tile([REG, 128], b16)        # my bf16 h shard (+zero row)
            h_cur = dramp2.tile([SP, 128], f32)        # fp32 h (this level)

            # ---- constants ----
            iota_t = constp.tile([128, 128], i16)
            nc.gpsimd.iota(iota_t[:], pattern=[[1, 128]], base=0,
                           channel_multiplier=0)
            ones_b = constp.tile([1, 128], b16)
            nc.vector.memset(ones_b[:], 1.0)
            ones_f = constp.tile([1, 128], f32)
            nc.vector.memset(ones_f[:], 1.0)
            zrow = constp.tile([1, 128], b16)
            nc.vector.memset(zrow[:], 0.0)
            nc.sync.dma_start(shard[SP:SP + 1, :], zrow[:])

            w_n2l_t = constp.tile([128, 128], f32)
            nc.sync.dma_start(w_n2l_t[:], w_n2l_d[:])
            b_n2l_t = constp.tile([1, 128], f32)
            nc.sync.dma_start(b_n2l_t[:], b_n2l_d[:])

            # ---- embed: h0 = tanh(nf @ w_n2l + b_n2l) ----
            for w in range(W):
                nf_w = smallp.tile([128, 128], f32, tag="nf")
                nc.sync.dma_start(nf_w[:], nf_t[:, w * 128:(w + 1) * 128])
                ps = psum_hn_pool.tile([128, 128], f32, tag="ps_hn", name="ps")
                nc.tensor.matmul(ps[:], nf_w[:], w_n2l_t[:], start=True, stop=False)
                nc.tensor.matmul(ps[:], ones_f[:], b_n2l_t[:], start=False, stop=True)
                h_w = smallp.tile([128, 128], f32, tag="hnew")
                nc.scalar.activation(h_w[:], ps[:], Tanh)
                hb = smallp.tile([128, 128], b16, tag="hb16")
                nc.vector.tensor_copy(hb[:], h_w[:])
                nc.sync.dma_start(h_cur[w * 128:(w + 1) * 128, :], h_w[:])
                nc.sync.dma_start(shard[w * 128:(w + 1) * 128, :], hb[:])

            # ---- level loop ----
            for lv in range(LV):
                hfull = dramp2.tile([NREG * 2 * REG, 128], b16, tag="hfull")
                nc.gpsimd.collective_compute(
                    "AllGather",
                    bass.mybir.AluOpType.bypass,
                    replica_groups=[list(range(NC))],
                    ins=[shard[:].opt()],
                    outs=[hfull[:].opt()],
                )

                convw_t = wtsp.tile([128, ET * 128], f32, tag="convw")
                nc.sync.dma_start(convw_t[:], convw_d[lv])
                convb_t = wtsp.tile([1, ET * 128], f32, tag="convb")
                nc.sync.dma_start(convb_t[:], convb_d[lv])
                mw_t = wtsp.tile([128, ET * 128], f32, tag="mw")
                nc.sync.dma_start(mw_t[:], mw_d[lv])
                mb_t = wtsp.tile([1, 128], f32, tag="mb")
                nc.sync.dma_start(mb_t[:], mb_d[lv])

                h_nxt = (dramp2.tile([SP, 128], f32, tag="h_cur", name="h_nxt")
                         if lv < LV - 1 else None)

                # gather stream state
                cur_piece = [-1] * NREG
                cur_g = [None] * NREG
                cur_p = [None] * NREG

                def ensure_piece(scp, pi):
                    if cur_piece[scp] == pi:
                        return
                    assert pi == cur_piece[scp] + 1 or cur_piece[scp] == -1
                    gpid, off0, n = piece_of_stream[scp][pi]
                    nch = n // 128
                    it = idxp.tile([128, idx_cols], i16, tag=f"idx{scp}")
                    nc.sync.dma_start(it[:, : n // 16], idx_d[gpid][:, : n // 16])
                    ot = idxp.tile([128, off_cols], i16, tag=f"off{scp}")
                    nc.sync.dma_start(ot[:, :nch], off_d[gpid][:, :nch])
                    gt = gatherp.tile([128, cfg.PIECE_CH, 128], b16, tag=f"g{scp}")
                    reg_base = scp * cfg.REG_ROWS
                    nc.gpsimd.dma_gather(
                        out_ap=gt[:, :nch, :],
                        in_ap=hfull[reg_base:reg_base + cfg.REG_ROWS, :],
                        idxs_ap=it[:, : n // 16],
                        num_idxs=n,
                        num_idxs_reg=n,
                        elem_size=128,
                        single_packet=False,
                    )
                    pt = ponep.tile([128, cfg.PIECE_CH * 128], b16, tag=f"p{scp}")
                    nc.vector.tensor_tensor(
                        out=pt[:, : nch * 128].rearrange("p (c e) -> p c e", e=128),
                        in0=ot[:, :nch].unsqueeze(2).to_broadcast([128, nch, 128]),
                        in1=iota_t[:].unsqueeze(1).to_broadcast([128, nch, 128]),
                        op=bass.mybir.AluOpType.is_equal,
                    )
                    cur_piece[scp] = pi
                    cur_g[scp] = gt
                    cur_p[scp] = pt

                for w in range(W):
                    mtanhs = []
                    for e in range(ET):
                        nchunks = int(K[:, :, w][e].sum())
                        if nchunks == 0:
                            continue
                        ps_s = psum_s_pool.tile([128, 128], f32)
                        ci_count = 0
                        for scp in range(NREG):
                            kk = int(K[e, scp, w])
                            if kk == 0:
                                continue
                            gc0 = int(group_slot_start[e, scp, w]) // 128
                            for k in range(kk):
                                gc = gc0 + k
                                pi = gc // cfg.PIECE_CH
                                ci = gc % cfg.PIECE_CH
                                ensure_piece(scp, pi)
                                nc.tensor.matmul(
                                    ps_s[:],
                                    cur_g[scp][:, ci, :],
                                    cur_p[scp][:, ci * 128:(ci + 1) * 128],
                                    start=(ci_count == 0),
                                    stop=(ci_count == nchunks - 1),
                                )
                                ci_count += 1
                        sT = smallp.tile([128, 128], f32, tag="sT")
                        nc.scalar.activation(sT[:], ps_s[:], Copy)
                        ps_t = psum_t_pool.tile([128, 128], f32)
                        nc.tensor.matmul(
                            ps_t[:], convw_t[:, e * 128:(e + 1) * 128], sT[:],
                            start=True, stop=False)
                        deg_t = smallp.tile([1, 128], f32, tag="deg", bufs=4,
                                            name="deg_t")
                        nc.sync.dma_start(deg_t[:], deg_d[e, w])
                        nc.tensor.matmul(
                            ps_t[:], convb_t[:, e * 128:(e + 1) * 128],
                            deg_t[:],
                            start=False, stop=True)
                        mt = mtp.tile([128, 128], f32, tag=f"mt{e}")
                        nc.scalar.activation(mt[:], ps_t[:], Tanh)
                        mtanhs.append((e, mt))

                    ps_hn = psum_hn_pool.tile([128, 128], f32, tag="ps_hn",
                                              name="ps_hn")
                    for i, (e, mt) in enumerate(mtanhs):
                        nc.tensor.matmul(
                            ps_hn[:], mt[:], mw_t[:, e * 128:(e + 1) * 128],
                            start=(i == 0), stop=False)
                    nc.tensor.matmul(ps_hn[:], ones_f[:], mb_t[:],
                                     start=(len(mtanhs) == 0), stop=True)
                    h_w = smallp.tile([128, 128], f32, tag="hres")
                    nc.sync.dma_start(h_w[:], h_cur[w * 128:(w + 1) * 128, :])
                    tmp = smallp.tile([128, 128], f32, tag="tmp")
                    nc.vector.tensor_tensor(out=tmp[:], in0=ps_hn[:], in1=h_w[:],
                                            op=bass.mybir.AluOpType.add)
                    hnew = smallp.tile([128, 128], f32, tag="hnew")
                    nc.scalar.activation(hnew[:], tmp[:], Tanh)
                    if lv < LV - 1:
                        nc.sync.dma_start(h_nxt[w * 128:(w + 1) * 128, :], hnew[:])
                        hb = smallp.tile([128, 128], b16, tag="hb16")
                        nc.vector.tensor_copy(hb[:], hnew[:])
                        nc.sync.dma_start(shard[w * 128:(w + 1) * 128, :], hb[:])
                    else:
                        nc.sync.dma_start(out_d[w * 128:(w + 1) * 128, :], hnew[:])
                if lv < LV - 1:
                    h_cur = h_nxt

    nc.compile()
    return nc


# ----------------------------------------------------------------------------
# weight packing (shared across cores)
# ----------------------------------------------------------------------------


def pack_weights(cfg, w_n2l, b_n2l, conv_w, conv_b, merge_w, merge_b):
    LV, ET = cfg.LV, cfg.ET
    packed = {
        "w_n2l": np.asarray(w_n2l, np.float32),
        "b_n2l": np.asarray(b_n2l, np.float32).reshape(1, 128),
        "conv_w": np.asarray(conv_w, np.float32),
        "conv_b": np.asarray(conv_b, np.float32).reshape(LV, 1, ET * 128),
        "merge_w": np.ascontiguousarray(
            np.asarray(merge_w, np.float32)
            .reshape(LV, ET, 128, 128).transpose(0, 2, 1, 3)
            .reshape(LV, 128, ET * 128)),
        "merge_b": np.asarray(merge_b, np.float32).reshape(LV, 1, 128),
    }
    return packed


def make_in_maps(cfg, node_feat, idx_arrs, off_arrs, deg_arrs, packed):
    in_maps = []
    nf = np.asarray(node_feat, np.float32)
    for c in range(cfg.NC):
        shard = np.zeros((cfg.SP, 128), np.float32)
        shard[: cfg.S] = nf[c * cfg.S:(c + 1) * cfg.S]
        m = {
            "node_feat_t": np.ascontiguousarray(shard.T),
            "idx": idx_arrs[c],
            "dstoff": off_arrs[c],
            "deg": deg_arrs[c],
        }
        m.update(packed)
        in_maps.append(m)
    return in_maps


# ----------------------------------------------------------------------------
# entry point
# ----------------------------------------------------------------------------


def kernel(node_feat, edge_src, edge_dst, w_n2l, b_n2l, conv_w, conv_b,
           merge_w, merge_b):
    from concourse.bass_utils import run_bass_kernel_spmd

    cfg = Cfg()
    plan, idx_arrs, off_arrs, deg_arrs = preprocess_edges(cfg, edge_src, edge_dst)
    packed = pack_weights(cfg, w_n2l, b_n2l, conv_w, conv_b, merge_w, merge_b)
    in_maps = make_in_maps(cfg, node_feat, idx_arrs, off_arrs, deg_arrs, packed)
    nc = build_program(cfg, plan)
    res = run_bass_kernel_spmd(nc, in_maps, core_ids=list(range(cfg.NC)))
    out = np.empty((cfg.N, 128), np.float32)
    for c in range(cfg.NC):
        out[c * cfg.S:(c + 1) * cfg.S] = res.results[c]["out"][: cfg.S]
    return out

